# revision 17
# baseline (speedup 1.0000x reference)
"""BiLSTM-CRF Trainium2 kernel.

Strategy (batch=1, S=8192 sequential recurrence shards poorly, so):
  - core 0 runs the forward LSTM, core 1 runs the backward LSTM (its inputs
    are reversed on the host), so the two sequential recurrences run in
    parallel.  Everything heavy is on-device:
      phase 1: embedding gather (indirect DMA, 128 rows/instr) -> PE
               transpose -> input projections x @ W_ih.T + b as fp16
               matmuls (fp32 PSUM) -> xp in DRAM
      phase 2: 8192-step LSTM recurrence.  W_hh stationary in fp16, h
               moving in fp16, PSUM/c/activations fp32.  64 matmuls of
               [128,128]x[128,1] per step produce the gates as [128,16]
               (gates on partitions), tail on DVE/ACT; o-gates computed
               last so the c-update overlaps them.  Dynamic For_i loop
               (U=16 steps/iteration) keeps trace/NEFF size bounded.
  - host: feats = [hf, hb] @ W_out.T + b_out, Viterbi + backtrace (tiny,
    O(S*K^2) work).

Numerics validated against the fp32 jax reference on the actual inputs:
fp16 recurrence gives 0/8192 Viterbi path flips, score rel-err 8.8e-8.

Perf notes (measured on the axon-tunneled TRN2 deployment): execution is
instruction-dispatch-bound (~1-3us per executed instruction regardless of
engine or size; N=512 matmuls cost the same as N=1).  U=64 bodies run 2x
slower than U=16 (I$ thrash), staggered_reset is neutral, so U=16 plain is
the shipped configuration.
"""

import sys

for _p in (
    "/root/.axon_site",
    "/root/.axon_site/_ro/trn_rl_repo",
    "/root/.axon_site/_ro/pypackages",
):
    if _p not in sys.path:
        sys.path.append(_p)

from contextlib import ExitStack

import numpy as np

import concourse.bacc as bacc
import concourse.bass as bass
import concourse.tile as tile
from concourse import mybir
from concourse.bass_utils import run_bass_kernel_spmd

V, E, HH, G = 50257, 512, 512, 2048
KTAGS, START, STOP, NEG = 5, 3, 4, -10000.0
S_FULL = 8192

F32 = mybir.dt.float32
F32R = mybir.dt.float32r
F16 = mybir.dt.float16
U32 = mybir.dt.uint32
AF = mybir.ActivationFunctionType

_BUILD_CACHE = {}


def _build(S, U, SB, trace_sim=False, variant="full", staggered=False):
    """Build + compile the 2-core SPMD program. Returns the Bacc object.

    variant: "full" | "mm_only" (no tail, PE stream only) | "no_mm"
    (tail only) — debug variants for component timing.
    """
    key = (S, U, SB, variant, staggered)
    if key in _BUILD_CACHE:
        return _BUILD_CACHE[key]
    nsb = S // SB
    assert SB % U == 0 and S % SB == 0 and S % 128 == 0

    nc = bacc.Bacc("TRN2", target_bir_lowering=False, debug=False, num_devices=2)

    FR = mybir.dt.bfloat16 if variant.endswith("bf16") else F16
    embed_d = nc.dram_tensor("embed", [V, E], F32, kind="ExternalInput").ap()
    tokens_d = nc.dram_tensor("tokens", [S], U32, kind="ExternalInput").ap()
    wih_d = nc.dram_tensor("wih_t", [4, 128, G], FR, kind="ExternalInput").ap()
    whh_d = nc.dram_tensor("whh_t", [4, 128, G], FR, kind="ExternalInput").ap()
    bias_d = nc.dram_tensor("bias", [G], F32, kind="ExternalInput").ap()
    h0_d = nc.dram_tensor("h0", [HH], F32, kind="ExternalInput").ap()
    c0_d = nc.dram_tensor("c0", [HH], F32, kind="ExternalInput").ap()
    ident_d = nc.dram_tensor("ident", [128, 128], F32, kind="ExternalInput").ap()
    hs_d = nc.dram_tensor("hs", [4, 128, S], F32, kind="ExternalOutput").ap()
    xp_d = nc.dram_tensor("xp", [16, 128, S], F32).ap()

    with tile.TileContext(nc, trace_sim=trace_sim) as tc:
        with ExitStack() as ctx:
            const_pool = ctx.enter_context(tc.tile_pool(name="const", bufs=1))
            state_pool = ctx.enter_context(tc.tile_pool(name="state", bufs=1))

            bias_sb = const_pool.tile([128, 16], F32)
            nc.sync.dma_start(bias_sb[:], bias_d.rearrange("(m p) -> p m", p=128))
            tok_sb = const_pool.tile([128, S // 128], U32)
            nc.sync.dma_start(tok_sb[:], tokens_d.rearrange("(b p) -> p b", p=128))
            ident_sb = const_pool.tile([128, 128], F32)
            nc.sync.dma_start(ident_sb[:], ident_d)

            h_sb = state_pool.tile([128, 4], FR)
            c_sb = state_pool.tile([128, 4], F32)
            h0t = state_pool.tile([128, 4], F32)
            nc.sync.dma_start(h0t[:], h0_d.rearrange("(k p) -> p k", p=128))
            nc.sync.dma_start(c_sb[:], c0_d.rearrange("(k p) -> p k", p=128))
            nc.vector.tensor_copy(h_sb[:], h0t[:])

            # ---------------- phase 1: gather + input projections ----------
            with ExitStack() as p1:
                wih_pool = p1.enter_context(tc.tile_pool(name="wih", bufs=1))
                xg_pool = p1.enter_context(tc.tile_pool(name="xg", bufs=3))
                xt_pool = p1.enter_context(tc.tile_pool(name="xt", bufs=2))
                pst_pool = p1.enter_context(
                    tc.tile_pool(name="pst", bufs=2, space="PSUM")
                )
                psx_pool = p1.enter_context(
                    tc.tile_pool(name="psx", bufs=2, space="PSUM")
                )
                xps_pool = p1.enter_context(tc.tile_pool(name="xps", bufs=3))

                wih_sb = wih_pool.tile([128, 4, G], FR)
                nc.sync.dma_start(wih_sb[:], wih_d.rearrange("e p j -> p e j"))

                for sb in range(nsb):
                    t0 = sb * SB
                    xt = xt_pool.tile([128, 4, SB], FR)
                    for tb in range(SB // 128):
                        xg = xg_pool.tile([128, E], F32)
                        bi = t0 // 128 + tb
                        nc.gpsimd.indirect_dma_start(
                            out=xg[:],
                            out_offset=None,
                            in_=embed_d[:],
                            in_offset=bass.IndirectOffsetOnAxis(
                                ap=tok_sb[:, bi : bi + 1], axis=0
                            ),
                        )
                        for ec in range(4):
                            pt = pst_pool.tile([128, 128], F32)
                            nc.tensor.transpose(
                                pt[:], xg[:, ec * 128 : (ec + 1) * 128], ident_sb[:]
                            )
                            nc.vector.tensor_copy(
                                xt[:, ec, tb * 128 : (tb + 1) * 128], pt[:]
                            )
                    for mc in range(16):
                        px = psx_pool.tile([128, SB], F32)
                        for ec in range(4):
                            nc.tensor.matmul(
                                px[:],
                                lhsT=wih_sb[:, ec, mc * 128 : (mc + 1) * 128],
                                rhs=xt[:, ec, :],
                                start=(ec == 0),
                                stop=(ec == 3),
                            )
                        xs = xps_pool.tile([128, SB], F32)
                        nc.scalar.activation(
                            xs[:], px[:], AF.Identity, bias=bias_sb[:, mc : mc + 1]
                        )
                        nc.sync.dma_start(xp_d[mc, :, t0 : t0 + SB], xs[:])

            # ---------------- phase 2: LSTM recurrence ---------------------
            with ExitStack() as p2:
                whh_pool = p2.enter_context(tc.tile_pool(name="whh", bufs=1))
                xp2_pool = p2.enter_context(tc.tile_pool(name="xp2", bufs=2))
                hsb_pool = p2.enter_context(tc.tile_pool(name="hsb", bufs=2))
                psg_pool = p2.enter_context(
                    tc.tile_pool(name="psg", bufs=2, space="PSUM")
                )
                psn_pool = p2.enter_context(
                    tc.tile_pool(name="psn", bufs=2, space="PSUM")
                )
                pso_pool = p2.enter_context(
                    tc.tile_pool(name="pso", bufs=2, space="PSUM")
                )
                tail_pool = p2.enter_context(tc.tile_pool(name="tail", bufs=2))

                whh_sb = whh_pool.tile([128, 4, G], FR)
                nc.sync.dma_start(whh_sb[:], whh_d.rearrange("e p j -> p e j"))

                for sb in range(nsb):
                    t0 = sb * SB
                    xp2 = xp2_pool.tile([128, 16, SB], F32)
                    nc.sync.dma_start(
                        xp2[:], xp_d.rearrange("m p t -> p m t")[:, :, t0 : t0 + SB]
                    )
                    hsb = hsb_pool.tile([128, 4, SB], F32)
                    if variant.startswith("mm_"):
                        nc.gpsimd.memset(hsb[:], 0.0)
                    with tc.For_i(
                        0,
                        SB // U,
                        1,
                        hint_engines=(mybir.EngineType.PE,),
                        staggered_reset=staggered,
                    ) as iv:
                        for u in range(U):
                            tloc = iv * U + u
                            if variant not in ("no_mm", "mm_n512"):
                                psg = psg_pool.tile([128, 12], F32)
                                pso = pso_pool.tile([128, 4], F32)
                            if not variant.startswith("mm_"):
                                gsb = tail_pool.tile([128, 16], F32, tag="gsb")
                                asb = tail_pool.tile([128, 16], F32, tag="asb")
                                tmp = tail_pool.tile([128, 12], F32, tag="tmp")
                            # gates i,f,g  (columns 0:12 of [128,16] layout)
                            if variant == "mm_n512":
                                psn = psn_pool.tile([128, 512], F32)
                                for r in range(8):
                                    nc.tensor.matmul(
                                        psn[:],
                                        lhsT=whh_sb[:, 1, 0:128],
                                        rhs=whh_sb[:, 0, 0:512],
                                        start=(r == 0),
                                        stop=(r == 7),
                                    )
                            elif variant != "no_mm":
                                for mc in range(12):
                                    for kc in range(4):
                                        nc.tensor.matmul(
                                            psg[:, mc : mc + 1],
                                            lhsT=whh_sb[
                                                :, kc, mc * 128 : (mc + 1) * 128
                                            ],
                                            rhs=h_sb[:, kc : kc + 1],
                                            start=(kc == 0),
                                            stop=(kc == 3),
                                        )
                                for mc in range(12, 16):
                                    for kc in range(4):
                                        nc.tensor.matmul(
                                            pso[:, mc - 12 : mc - 11],
                                            lhsT=whh_sb[
                                                :, kc, mc * 128 : (mc + 1) * 128
                                            ],
                                            rhs=h_sb[:, kc : kc + 1],
                                            start=(kc == 0),
                                            stop=(kc == 3),
                                        )
                            if variant.startswith("mm_"):
                                continue
                            if variant == "no_mm":
                                nc.vector.tensor_copy(
                                    gsb[:, 0:12],
                                    xp2[:, 0:12, bass.ds(tloc, 1)].rearrange(
                                        "p m o -> p (m o)"
                                    ),
                                )
                            else:
                                nc.vector.tensor_add(
                                    gsb[:, 0:12],
                                    psg[:],
                                    xp2[:, 0:12, bass.ds(tloc, 1)].rearrange(
                                        "p m o -> p (m o)"
                                    ),
                                )
                            nc.scalar.activation(asb[:, 0:8], gsb[:, 0:8], AF.Sigmoid)
                            nc.scalar.activation(asb[:, 8:12], gsb[:, 8:12], AF.Tanh)
                            nc.vector.tensor_mul(
                                tmp[:, 0:4], asb[:, 0:4], asb[:, 8:12]
                            )  # i*g
                            nc.vector.tensor_mul(
                                tmp[:, 4:8], c_sb[:], asb[:, 4:8]
                            )  # c*f
                            nc.vector.tensor_add(c_sb[:], tmp[:, 0:4], tmp[:, 4:8])
                            nc.scalar.activation(tmp[:, 8:12], c_sb[:], AF.Tanh)
                            if variant == "no_mm":
                                nc.vector.tensor_copy(
                                    gsb[:, 12:16],
                                    xp2[:, 12:16, bass.ds(tloc, 1)].rearrange(
                                        "p m o -> p (m o)"
                                    ),
                                )
                            else:
                                nc.vector.tensor_add(
                                    gsb[:, 12:16],
                                    pso[:],
                                    xp2[:, 12:16, bass.ds(tloc, 1)].rearrange(
                                        "p m o -> p (m o)"
                                    ),
                                )
                            nc.scalar.activation(
                                asb[:, 12:16], gsb[:, 12:16], AF.Sigmoid
                            )
                            nc.vector.tensor_mul(
                                hsb[:, :, bass.ds(tloc, 1)].rearrange(
                                    "p k o -> p (k o)"
                                ),
                                asb[:, 12:16],
                                tmp[:, 8:12],
                            )
                            nc.vector.tensor_mul(
                                h_sb[:], asb[:, 12:16], tmp[:, 8:12]
                            )  # fp16 h for next step
                    nc.sync.dma_start(
                        hs_d.rearrange("k p t -> p k t")[:, :, t0 : t0 + SB], hsb[:]
                    )

    nc.compile()
    _BUILD_CACHE[key] = nc
    return nc


def _prep_core_inputs(tokens, embed, W_ih, W_hh, b, h0_row, c0_row, rdt=np.float16):
    wih_t = np.ascontiguousarray(
        W_ih.T.astype(np.float32).reshape(4, 128, G).astype(rdt)
    )  # [ec,128,G]; W_ih.T is [E, G]
    whh_t = np.ascontiguousarray(
        W_hh.T.astype(np.float32).reshape(4, 128, G).astype(rdt)
    )
    return {
        "embed": np.ascontiguousarray(embed, dtype=np.float32),
        "tokens": np.ascontiguousarray(tokens, dtype=np.uint32),
        "wih_t": wih_t,
        "whh_t": whh_t,
        "bias": np.ascontiguousarray(b, dtype=np.float32),
        "h0": np.ascontiguousarray(h0_row, dtype=np.float32),
        "c0": np.ascontiguousarray(c0_row, dtype=np.float32),
        "ident": np.eye(128, dtype=np.float32),
    }


def _viterbi_host(feats, transitions):
    Sn = feats.shape[0]
    fv = np.full(KTAGS, NEG, np.float32)
    fv[START] = 0.0
    bptrs = np.empty((Sn, KTAGS), np.int64)
    trans = transitions.astype(np.float32)
    for t in range(Sn):
        sc = fv[None, :] + trans
        bptrs[t] = np.argmax(sc, axis=1)
        fv = np.max(sc, axis=1) + feats[t]
    terminal = fv + trans[STOP]
    best = int(np.argmax(terminal))
    score = terminal[best]
    path = np.empty(Sn, np.int32)
    bl = bptrs.tolist()
    tag = best
    for t in range(Sn - 1, -1, -1):
        path[t] = tag
        tag = bl[t][tag]
    return np.float32(score), path


def _run_device(inputs, S, U=16, SB=512, trace=False):
    nc = _build(S, U, SB)
    sentence = np.asarray(inputs["sentence"]).astype(np.uint32)[:S]
    embed = np.asarray(inputs["embed"], dtype=np.float32)
    h0 = np.asarray(inputs["h0"], dtype=np.float32)
    c0 = np.asarray(inputs["c0"], dtype=np.float32)
    in0 = _prep_core_inputs(
        sentence,
        embed,
        np.asarray(inputs["W_ih_f"]),
        np.asarray(inputs["W_hh_f"]),
        np.asarray(inputs["b_f"]),
        h0[0],
        c0[0],
    )
    in1 = _prep_core_inputs(
        sentence[::-1],
        embed,
        np.asarray(inputs["W_ih_b"]),
        np.asarray(inputs["W_hh_b"]),
        np.asarray(inputs["b_b"]),
        h0[1],
        c0[1],
    )
    res = run_bass_kernel_spmd(nc, [in0, in1], [0, 1], trace=trace)
    hf = res.results[0]["hs"].reshape(HH, S).T
    hb = res.results[1]["hs"].reshape(HH, S).T[::-1]
    return hf, hb, res


def kernel(
    sentence,
    embed,
    W_ih_f,
    W_hh_f,
    b_f,
    W_ih_b,
    W_hh_b,
    b_b,
    W_out,
    b_out,
    transitions,
    h0,
    c0,
):
    inputs = dict(
        sentence=sentence,
        embed=embed,
        W_ih_f=W_ih_f,
        W_hh_f=W_hh_f,
        b_f=b_f,
        W_ih_b=W_ih_b,
        W_hh_b=W_hh_b,
        b_b=b_b,
        W_out=W_out,
        b_out=b_out,
        transitions=transitions,
        h0=h0,
        c0=c0,
    )
    S = int(np.asarray(sentence).shape[0])
    hf, hb, _ = _run_device(inputs, S)
    feats = (
        np.concatenate([hf, hb], axis=1) @ np.asarray(W_out, np.float32).T
        + np.asarray(b_out, np.float32)
    ).astype(np.float32)
    score, path = _viterbi_host(feats, np.asarray(transitions, np.float32))
    return score, path
